# revision 3
# baseline (speedup 1.0000x reference)
"""NTXEnt (intra-sample) loss kernel for Trainium2, 8-core data-parallel.

v3: TensorE Gram formulation with normalization folded into the PE
transposes.

Math (matches the jax reference):
  inp [C=8, V=2, B=4096, D=512] fp32
  xn = inp / max(||inp||_D, 1e-12)
  sim[i,b,jv] = <xn[i,0,b], xn[jv,b]> / T          (T = 0.1)
  loss[i,b]   = log(sum_{jv != (i,0)} exp(sim[i,b,jv])) - sim[i,b,(i,1)]
  answer = mean over (i, b).

Sharding: pure data parallel over B (4096 -> 8 cores x 512); per core 4
chunks of 128 batch rows.

Per chunk:
  1. SWDGE DMA-cast load: Xb [128 b, 16 iv, 512 d] bf16 (fp32 in HBM).
  2. nn[b,iv] = ||x||^2: STT self-dot w/ accumulate (split vector/scalar).
  3. r = exp(-.5 ln max(nn, eps^2)).
  4. diag_iv = ident * r[:, iv]  (per-partition tensor_scalar -> bf16).
  5. PE "transpose+normalize": XnT_k[d, iv, b] = Xb[:,iv,kwin]^T @ diag_iv
     (regular matmul: stationary = Xb slice, moving = diag(r)); 4 ivs per
     PSUM bank; contiguous [128, 512] PSUM->SBUF copies (split v/s).
  6. Gram groups g = {b : b % 16 == g} (stride-16 columns compose into a
     single-stride matmul operand): G = sum_k XnT_k[:, :, g::16]^T @ same.
     G[(iv,bl),(iv',bl')] = cos for bl==bl' (cross-b entries junk).
     4 G tiles share a PSUM bank.
  7. E = exp(10*G), one scalar activation per bank of 4 tiles, bf16 out.
  8. den[p] = sum_n E*maskd (vector STT w/ accumulate), maskd = same-bl
     block minus the self crop's view-0 column.
  9. posexp[p] = E[p, poscol(p)] via tensor_mask_reduce(op=max) with
     per-partition mask window.
  10. At kernel end (batched, one act-table visit): loss = ln(den_all) -
      ln(pex_all); host sums the even-iv rows / (C*B).

The scalar engine's activation table is pinned to
natural_log_exp_and_others (holds copy/square/exp/ln) to avoid per-chunk
ACT_TABLE_LOAD thrash.
"""

import os
import numpy as np

C, V, B, D = 8, 2, 4096, 512
NCORES = 8
B_LOC = B // NCORES            # 512
P = 128
NCHUNK = B_LOC // P            # 4
GRP = 8                        # b's per Gram group (stride-16)
NGRP = P // GRP                # 16 groups per chunk
CV = C * V                     # 16
KCH = D // P                   # 4 d-chunks
TEMP_INV = 10.0
EPS2 = 1e-24

# engine split knobs (per-chunk counts out of 16): how many norm STTs and
# PSUM->SBUF copies go on vector (rest on scalar)
N_NORM_V = int(os.environ.get("NTX_NORM_V", "8"))
N_COPY_V = int(os.environ.get("NTX_COPY_V", "8"))
N_DIAG_V = int(os.environ.get("NTX_DIAG_V", "16"))


def _iv_of(p):
    return p // GRP


def _bl_of(p):
    return p % GRP


def _make_consts():
    """maskd [128,128] bf16; pos start/end fp32 [128,1].

    Row/col index m = 8*iv + bl (iv-major).  Block: bl == bl'.
    Self col (excluded): iv' = 2*(iv//2) (the crop's view-0).
    Pos col: iv' = iv | 1 (view-1 of the same crop; junk for odd rows).
    """
    p = np.arange(P)
    iv, bl = p // GRP, p % GRP
    n = np.arange(P)
    ivn, bln = n // GRP, n % GRP
    same = bln[None, :] == bl[:, None]
    selfcol = (ivn[None, :] == (iv // 2 * 2)[:, None])
    maskd = (same & ~selfcol).astype(np.float32)
    poscol = GRP * (iv | 1) + bl
    maskp = (n[None, :] == poscol[:, None]).astype(np.float32)
    return maskd, maskp


def _build_kernel(b_loc=B_LOC):
    from contextlib import ExitStack

    import concourse.bacc as bacc
    import concourse.tile as tile
    import concourse.mybir as mybir
    from concourse.masks import make_identity

    # Pin every scalar-engine activation to the one table that holds
    # copy/square/exp/ln, preserving act_func_set_id indices.
    if not getattr(bacc, "_ntx_act_tables_pinned", False):
        _orig_gat = bacc.get_activation_tables

        def _pinned(arch):
            tabs = _orig_gat(arch)
            out = {}
            for name, funcs in tabs.items():
                out[name] = funcs if name == "natural_log_exp_and_others" \
                    else set()
            return out

        bacc.get_activation_tables = _pinned
        bacc._ntx_act_tables_pinned = True

    f32 = mybir.dt.float32
    bf16 = mybir.dt.bfloat16
    Alu = mybir.AluOpType
    Act = mybir.ActivationFunctionType

    nchunk = b_loc // P
    nc = bacc.Bacc("TRN2", target_bir_lowering=False, debug=False)
    x_d = nc.dram_tensor("inp", [C, V, b_loc, D], f32, kind="ExternalInput")
    md_d = nc.dram_tensor("maskd", [P, P], bf16, kind="ExternalInput")
    mp_d = nc.dram_tensor("maskp", [P, P], bf16, kind="ExternalInput")
    o_d = nc.dram_tensor("out", [P, 2 * nchunk * NGRP], f32,
                         kind="ExternalOutput")

    with tile.TileContext(nc) as tc, ExitStack() as ctx:
        const_p = ctx.enter_context(tc.tile_pool(name="const", bufs=1))
        xp = ctx.enter_context(tc.tile_pool(name="x", bufs=3))
        xtp = ctx.enter_context(tc.tile_pool(name="xt", bufs=3))
        dgp = ctx.enter_context(tc.tile_pool(name="dg", bufs=3))
        small = ctx.enter_context(tc.tile_pool(name="small", bufs=3))
        scr_v = ctx.enter_context(tc.tile_pool(name="scr_v", bufs=3))
        ep = ctx.enter_context(tc.tile_pool(name="ep", bufs=3))
        outp = ctx.enter_context(tc.tile_pool(name="outp", bufs=1))
        ps_t = ctx.enter_context(
            tc.tile_pool(name="ps_t", bufs=4, space="PSUM"))
        ps_g = ctx.enter_context(
            tc.tile_pool(name="ps_g", bufs=3, space="PSUM"))

        ident = const_p.tile([P, P], f32)
        make_identity(nc, ident[:, :])
        maskd = const_p.tile([P, P], bf16)
        nc.sync.dma_start(out=maskd[:, :], in_=md_d.ap())
        maskp = const_p.tile([P, P], bf16)
        nc.sync.dma_start(out=maskp[:, :], in_=mp_d.ap())

        den_all = outp.tile([P, nchunk * NGRP], f32)
        pex_all = outp.tile([P, nchunk * NGRP], f32)
        loss_out = outp.tile([P, 2 * nchunk * NGRP], f32)

        x_ap = x_d.ap()

        # software-pipelined 3-stage emission so each engine's in-order
        # queue never head-of-line blocks: front(c) computes r/diags while
        # mid(c-1) transposes and back(c-2) runs Grams + postprocessing.
        state = {}

        def stage_front(c):
            X = xp.tile([P, CV, D], bf16, name="X", tag="X")
            src = x_ap[:, :, c * P:(c + 1) * P, :].rearrange(
                "i v b d -> b (i v) d")
            Xsrc = X
            for h in range(4):
                nc.gpsimd.dma_start(out=X[:, 4 * h:4 * h + 4, :],
                                    in_=src[:, 4 * h:4 * h + 4, :])

            nn = small.tile([P, CV], f32, tag="nn", name="nn")
            for iv in range(CV):
                if iv < N_NORM_V:
                    scr = scr_v.tile([P, D], bf16, tag="sqv", name="sq")
                    nc.vector.scalar_tensor_tensor(
                        out=scr[:, :], in0=Xsrc[:, iv, :], scalar=1.0,
                        in1=Xsrc[:, iv, :], op0=Alu.mult, op1=Alu.mult,
                        accum_out=nn[:, iv:iv + 1])
                else:
                    scr = scr_v.tile([P, D], bf16, tag="sqs", name="sq")
                    nc.scalar.activation(
                        out=scr[:, :], in_=Xsrc[:, iv, :], func=Act.Square,
                        accum_out=nn[:, iv:iv + 1])

            # r-chain per iv-quad so diags/transposes unblock early
            nnc = small.tile([P, CV], f32, tag="nnc", name="nnc")
            lnn = small.tile([P, CV], f32, tag="lnn", name="lnn")
            r = small.tile([P, CV], f32, tag="r", name="r")
            for h in range(4):
                s = slice(4 * h, 4 * h + 4)
                nc.vector.tensor_scalar_max(nnc[:, s], nn[:, s], EPS2)
                nc.scalar.activation(out=lnn[:, s], in_=nnc[:, s],
                                     func=Act.Ln)
                nc.scalar.activation(out=r[:, s], in_=lnn[:, s],
                                     func=Act.Exp, scale=-0.5)

            diags = dgp.tile([P, CV, P], bf16, tag="diags", name="diags")
            for iv in range(CV):
                if iv < N_DIAG_V:
                    # diag(r_iv) on gpsimd: (p - n) == 0 ? r[p] : 0
                    rb = r[:, iv:iv + 1].broadcast_to([P, P])
                    nc.gpsimd.affine_select(
                        out=diags[:, iv, :], in_=rb,
                        compare_op=Alu.is_equal, fill=0.0,
                        base=0, pattern=[[-1, P]], channel_multiplier=1)
                else:
                    nc.vector.tensor_scalar_mul(
                        diags[:, iv, :], ident[:, :], r[:, iv:iv + 1])
            state[c] = {"X": X, "diags": diags}

        def stage_mid(c):
            X, diags = state[c]["X"], state[c]["diags"]
            XnT = [xtp.tile([P, NGRP, CV, GRP], bf16, tag=f"xt{k}",
                            name=f"XnT{k}")
                   for k in range(KCH)]
            ci = 0
            for k in range(KCH):
                for q in range(CV // 4):
                    tp = ps_t.tile([P, 4 * P], f32, tag="tp", name="tp")
                    for t in range(4):
                        iv = 4 * q + t
                        nc.tensor.matmul(
                            tp[:, t * P:(t + 1) * P],
                            X[:, iv, k * P:(k + 1) * P],
                            diags[:, iv, :],
                            start=True, stop=True)
                    # src cols: (t 4, g 16, bl 8); dst: [:, g, 4q+t, bl]
                    dst = XnT[k][:, :, 4 * q:4 * q + 4, :].rearrange(
                        "p g t b -> p t g b")
                    srcv = tp[:, :].rearrange("p (t g b) -> p t g b",
                                              t=4, g=NGRP)
                    if ci % CV < N_COPY_V:
                        nc.vector.tensor_copy(out=dst, in_=srcv)
                    else:
                        nc.scalar.copy(out=dst, in_=srcv)
                    ci += 1
            state[c]["XnT"] = XnT

        def stage_back(c):
            XnT = state.pop(c)["XnT"]
            for gq in range(NGRP // 4):
                Gb = ps_g.tile([P, 4 * P], f32, tag="Gb", name="Gb")
                for j in range(4):
                    g = 4 * gq + j
                    for k in range(KCH):
                        opk = XnT[k][:, g, :, :].rearrange(
                            "p iv b -> p (iv b)")
                        nc.tensor.matmul(
                            Gb[:, j * P:(j + 1) * P], opk, opk,
                            start=(k == 0), stop=(k == KCH - 1))
                E = ep.tile([P, 4 * P], bf16, tag="E", name="E")
                nc.scalar.activation(out=E[:, :], in_=Gb[:, :],
                                     func=Act.Exp, scale=TEMP_INV)
                # denominators / positives for 4 tiles: masked TT then
                # segmented reduce (mask broadcast over the tile axis)
                col = c * NGRP + 4 * gq
                Ev = E[:, :].rearrange("p (t n) -> p t n", t=4)
                mdb = maskd[:, :].unsqueeze(1).broadcast_to([P, 4, P])
                sv = scr_v.tile([P, 4 * P], bf16, tag="sttv", name="sv")
                svv = sv[:, :].rearrange("p (t n) -> p t n", t=4)
                nc.vector.tensor_tensor(out=svv, in0=Ev, in1=mdb,
                                        op=Alu.mult)
                nc.vector.tensor_reduce(
                    out=den_all[:, col:col + 4], in_=svv,
                    axis=mybir.AxisListType.X, op=Alu.add)
                mpb = maskp[:, :].unsqueeze(1).broadcast_to([P, 4, P])
                mr = scr_v.tile([P, 4 * P], bf16, tag="mrv", name="mr")
                mrv = mr[:, :].rearrange("p (t n) -> p t n", t=4)
                nc.vector.tensor_tensor(out=mrv, in0=Ev, in1=mpb,
                                        op=Alu.mult)
                nc.vector.tensor_reduce(
                    out=pex_all[:, col:col + 4], in_=mrv,
                    axis=mybir.AxisListType.X, op=Alu.add)

        for step in range(nchunk + 2):
            if step < nchunk:
                stage_front(step)
            if 1 <= step <= nchunk:
                stage_mid(step - 1)
            if step >= 2:
                stage_back(step - 2)

        # ---- loss = ln(den) - ln(posexp), batched
        ncols = nchunk * NGRP
        lnD = loss_out[:, 0:ncols]
        lnP = loss_out[:, ncols:2 * ncols]
        nc.scalar.activation(out=lnD, in_=den_all[:, :], func=Act.Ln)
        nc.scalar.activation(out=lnP, in_=pex_all[:, :], func=Act.Ln)
        nc.sync.dma_start(out=o_d.ap(), in_=loss_out[:, :])

    nc.compile()
    return nc


_CACHE = {}


def _get_nc(b_loc=B_LOC):
    key = (b_loc, N_NORM_V, N_COPY_V, N_DIAG_V)
    if key not in _CACHE:
        _CACHE[key] = _build_kernel(b_loc)
    return _CACHE[key]


def _run(inp, trace=False):
    from concourse.bass_utils import run_bass_kernel_spmd

    nc = _get_nc()
    import ml_dtypes
    maskd, maskp = _make_consts()
    maskd = maskd.astype(ml_dtypes.bfloat16)
    maskp = maskp.astype(ml_dtypes.bfloat16)
    # b-permutation within each 128-chunk: partition p = (g, bl) holds
    # b = chunk*128 + 16*(p % 8) + p // 8
    p = np.arange(P)
    perm = 16 * (p % GRP) + p // GRP
    full_perm = (np.arange(B_LOC).reshape(NCHUNK, P)[:, 0] [:, None]
                 + perm[None, :]).reshape(-1)
    in_maps = []
    for k in range(NCORES):
        shard = inp[:, :, k * B_LOC:(k + 1) * B_LOC, :][:, :, full_perm, :]
        shard = np.ascontiguousarray(shard, dtype=np.float32)
        in_maps.append({"inp": shard, "maskd": maskd, "maskp": maskp})
    res = run_bass_kernel_spmd(nc, in_maps, list(range(NCORES)), trace=trace)
    ncols = NCHUNK * NGRP
    # anchor rows: even iv -> (p // 8) % 2 == 0
    p = np.arange(P)
    anchor = (p // GRP) % 2 == 0
    total = np.float64(0.0)
    for m in res.results:
        o = m["out"].astype(np.float64)
        loss_rows = o[:, 0:ncols] - o[:, ncols:2 * ncols]
        total += loss_rows[anchor, :].sum()
    loss = np.float32(total / (C * B))
    return loss, res


def kernel(inp):
    inp = np.asarray(inp)
    for _ in range(3):
        loss, _ = _run(inp, trace=False)
        if np.isfinite(loss):
            return loss
    return loss
